# revision 5
# baseline (speedup 1.0000x reference)
"""Gemma3 decoder layer on 8 Trainium2 NeuronCores (Bass/Tile).

Sharding (per core c):
  - attention: tensor-parallel over heads; core c owns Q head c, KV head c//2.
  - wo: replicated weight, token-sharded rows (core c computes tokens [256c, 256c+256)).
  - MLP: gate/up column-sharded (1280 cols each), down row-sharded; partial
    sums combined with a ReduceScatter that lands each core its token shard.
  - norms/residual: token-sharded.
Dataflow: in_ln -> AG(h^T) -> QKV+rope+attn -> A2A(attn^T) -> wo+norms ->
  AG(h2^T) -> gate/up/down -> RS -> post_ff+residual.
Matmuls run in bf16 (fp32 PSUM accumulation); norms/softmax/residual in fp32.
"""
import sys

if "/opt/trn_rl_repo" not in sys.path:
    sys.path.insert(0, "/opt/trn_rl_repo")

import numpy as np
import ml_dtypes

import concourse.bass as bass
import concourse.mybir as mybir
import concourse.tile as tile
from concourse import bacc
from concourse.bass_utils import run_bass_kernel_spmd
from concourse.masks import make_identity

dt = mybir.dt
AF = mybir.ActivationFunctionType
ALU = mybir.AluOpType
BF = dt.bfloat16
F32 = dt.float32

HID, NH, NKV, HD, INTER = 2560, 8, 4, 256, 10240
WIN, EPS, BASE = 512, 1e-6, 10000.0
S = 2048
NC_ = 8
TS = S // NC_              # 256 tokens per core
KH = HID // 128            # 20 hidden-dim chunks
KA = (NH * HD) // 128      # 16 attn-dim chunks
MI = INTER // NC_ // 128   # 10 inter m-tiles per core
HALF = HD // 2


def _bcast_row(nc, sbuf_tile, dram_t, width):
    a = dram_t.ap()
    nc.sync.dma_start(sbuf_tile[:], bass.AP(
        tensor=a.tensor, offset=a.offset, ap=[[0, 128], [1, width]]))


def _swap_ap(t, w):
    """Read tile t [128, 2w] with free-dim halves swapped (as [128,2,w])."""
    a = t[:, 0:2 * w]
    return bass.AP(tensor=a.tensor, offset=a.offset + w,
                   ap=[list(a.ap[0]), [-w, 2], [1, w]])


def build_nc():
    nc = bacc.Bacc("TRN2", target_bir_lowering=False, debug=False,
                   enable_asserts=True, num_devices=NC_)

    x_shard = nc.dram_tensor("x_shard", [TS, HID], F32, kind="ExternalInput")
    wq_c = nc.dram_tensor("wq_c", [HID, HD], BF, kind="ExternalInput")
    wk_c = nc.dram_tensor("wk_c", [HID, HD], BF, kind="ExternalInput")
    wv_c = nc.dram_tensor("wv_c", [HID, HD], BF, kind="ExternalInput")
    wo_f = nc.dram_tensor("wo_f", [NH * HD, HID], BF, kind="ExternalInput")
    wg_c = nc.dram_tensor("wg_c", [HID, INTER // NC_], BF, kind="ExternalInput")
    wu_c = nc.dram_tensor("wu_c", [HID, INTER // NC_], BF, kind="ExternalInput")
    wd_c = nc.dram_tensor("wd_c", [INTER // NC_, HID], BF, kind="ExternalInput")
    w1_in = nc.dram_tensor("w1_in", [HID], BF, kind="ExternalInput")
    w1_pa = nc.dram_tensor("w1_pa", [HID], BF, kind="ExternalInput")
    w1_pf = nc.dram_tensor("w1_pf", [HID], BF, kind="ExternalInput")
    w1_po = nc.dram_tensor("w1_po", [HID], F32, kind="ExternalInput")
    cqw = nc.dram_tensor("cqw", [S, HD], BF, kind="ExternalInput")
    sqw = nc.dram_tensor("sqw", [S, HD], BF, kind="ExternalInput")
    ckw = nc.dram_tensor("ckw", [S, HD], BF, kind="ExternalInput")
    skw = nc.dram_tensor("skw", [S, HD], BF, kind="ExternalInput")
    out_shard = nc.dram_tensor("out_shard", [TS, HID], F32, kind="ExternalOutput")

    rg = [list(range(NC_))]

    with tile.TileContext(nc) as tc:
        with (
            tc.tile_pool(name="dram", bufs=1, space="DRAM") as dram,
            tc.tile_pool(name="glob", bufs=1) as glob,
            tc.tile_pool(name="nrm", bufs=3) as nrm,
            tc.tile_pool(name="psP", bufs=1, space="PSUM") as psP,
        ):
            # DRAM scratch
            hT_in = dram.tile([HID, TS], BF)
            hT_full = dram.tile([NC_ * HID, TS], BF, addr_space="Shared")
            a2a_in = dram.tile([S, TS], BF)
            a2a_out = dram.tile([S, TS], BF)
            h2T_in = dram.tile([HID, TS], BF)
            h2T_full = dram.tile([NC_ * HID, TS], BF, addr_space="Shared")
            rs_in = dram.tile([S, HID], BF)
            rs_out = dram.tile([TS, HID], BF)
            x2_spill = dram.tile([TS, HID], F32)

            ident = glob.tile([128, 128], BF)
            make_identity(nc, ident[:])
            eps_t = glob.tile([128, 1], F32)
            nc.vector.memset(eps_t[:], EPS)

            def rmsnorm_rinv(src_ap, d, name):
                """rinv[p,1]=1/sqrt(mean(src^2)+EPS) via bn_stats + ln/exp."""
                nsub = max(1, d // 512)
                stats = nrm.tile([128, nsub, 6], F32, tag="nst", name=f"{name}_st")
                if nsub > 1:
                    view = src_ap.rearrange("p (s f) -> p s f", s=nsub)
                    for i in range(nsub):
                        nc.vector.bn_stats(out=stats[:, i, :], in_=view[:, i, :])
                else:
                    nc.vector.bn_stats(out=stats[:, 0, :], in_=src_ap)
                mv = nrm.tile([128, 2], F32, tag="nmv", name=f"{name}_mv")
                nc.vector.bn_aggr(out=mv[:], in_=stats[:])
                ms = nrm.tile([128, 1], F32, tag="nms", name=f"{name}_ms")
                nc.vector.scalar_tensor_tensor(ms[:], mv[:, 0:1], mv[:, 0:1],
                                               mv[:, 1:2], op0=ALU.mult, op1=ALU.add)
                lnm = nrm.tile([128, 1], F32, tag="nln", name=f"{name}_ln")
                nc.scalar.activation(lnm[:], ms[:], AF.Ln, bias=eps_t[:])
                rinv = nrm.tile([128, 1], F32, tag="nrv", name=f"{name}_rv")
                nc.scalar.activation(rinv[:], lnm[:], AF.Exp, scale=-0.5)
                return rinv

            with tc.tile_pool(name="xpool", bufs=1) as xpool:
                x_sb = [xpool.tile([128, HID], F32, name=f"xt{t}") for t in range(2)]
                h16s = [None, None]

                # ============ S1: in_ln + transpose + AG1 ============
                with tc.tile_pool(name="s1", bufs=2) as s1:
                    w1_in_b = s1.tile([128, HID], BF, bufs=1)
                    _bcast_row(nc, w1_in_b, w1_in, HID)
                    for t in range(2):
                        nc.sync.dma_start(x_sb[t][:],
                                          x_shard.ap()[t * 128:(t + 1) * 128, :])
                        rinv = rmsnorm_rinv(x_sb[t][:], HID, f"inln{t}")
                        h16 = s1.tile([128, HID], BF, tag="h16", name=f"h16_{t}", bufs=2)
                        nc.vector.scalar_tensor_tensor(h16[:], x_sb[t][:], rinv[:],
                                                       w1_in_b[:], op0=ALU.mult,
                                                       op1=ALU.mult)
                        h16s[t] = h16
                    for k in range(KH):
                        hTk = s1.tile([128, TS], BF, tag="hTk", name=f"hTk{k}", bufs=3)
                        for t in range(2):
                            ptr = psP.tile([128, 128], BF, tag="tr", bufs=2,
                                           name=f"s1tr{k}_{t}")
                            nc.tensor.transpose(
                                ptr[:], h16s[t][:, k * 128:(k + 1) * 128], ident[:])
                            nc.vector.tensor_copy(hTk[:, t * 128:(t + 1) * 128], ptr[:])
                        nc.sync.dma_start(hT_in[k * 128:(k + 1) * 128, :], hTk[:])
                    nc.gpsimd.collective_compute(
                        "AllGather", ALU.bypass, replica_groups=rg,
                        ins=[hT_in[:]], outs=[hT_full[:]])

                # ============ S2/S3: attention ============
                with tc.tile_pool(name="attp", bufs=1) as attp:
                    QT = [attp.tile([128, S], BF, name=f"QT{h}") for h in range(2)]
                    KT = [attp.tile([128, S], BF, name=f"KT{h}") for h in range(2)]
                    V = [attp.tile([128, HD + 1], BF, name=f"V{i}")
                         for i in range(S // 128)]
                    aT_sb = [attp.tile([128, S], BF, name=f"aT{h}") for h in range(2)]
                    masks = attp.tile([128, 8, 512], BF)
                    for i in range(8):
                        delta = 512 - 128 * i
                        mk = masks[:, i, :]
                        nc.gpsimd.memset(mk, 1.0)
                        nc.gpsimd.affine_select(
                            out=mk, in_=mk, compare_op=ALU.is_ge, fill=0.0,
                            base=delta, pattern=[[1, 512]], channel_multiplier=-1)
                        nc.gpsimd.affine_select(
                            out=mk, in_=mk, compare_op=ALU.is_ge, fill=0.0,
                            base=-delta + (WIN - 1), pattern=[[-1, 512]],
                            channel_multiplier=1)
                    hT_v = hT_full[:].rearrange("(r k p) t -> r p k t", r=NC_, p=128)

                    with tc.tile_pool(name="s2", bufs=2) as s2:
                        wq_sb = s2.tile([128, KH, HD], BF, bufs=1)
                        wk_sb = s2.tile([128, KH, HD], BF, bufs=1)
                        wv_sb = s2.tile([128, KH, HD], BF, bufs=1)
                        for wsb, wdr in ((wq_sb, wq_c), (wk_sb, wk_c), (wv_sb, wv_c)):
                            nc.sync.dma_start(
                                wsb[:], wdr.ap().rearrange("(k p) n -> p k n", p=128))
                        for tt in range(S // 128):
                            r, u = tt // 2, tt % 2
                            hTtt = s2.tile([128, KH, 128], BF, tag="hTtt",
                                           name=f"hTtt{tt}", bufs=3)
                            nc.sync.dma_start(hTtt[:],
                                              hT_v[r, :, :, u * 128:(u + 1) * 128])
                            pq = psP.tile([128, HD], F32, tag="mm", bufs=6,
                                          name=f"pq{tt}")
                            pk = psP.tile([128, HD], F32, tag="mm", bufs=6,
                                          name=f"pk{tt}")
                            pv = psP.tile([128, HD], F32, tag="mm", bufs=6,
                                          name=f"pv{tt}")
                            for k in range(KH):
                                st, sp = (k == 0), (k == KH - 1)
                                nc.tensor.matmul(pq[:], hTtt[:, k, :], wq_sb[:, k, :],
                                                 start=st, stop=sp)
                                nc.tensor.matmul(pk[:], hTtt[:, k, :], wk_sb[:, k, :],
                                                 start=st, stop=sp)
                                nc.tensor.matmul(pv[:], hTtt[:, k, :], wv_sb[:, k, :],
                                                 start=st, stop=sp)
                            nc.vector.memset(V[tt][:, HD:HD + 1], 1.0)
                            nc.vector.tensor_copy(V[tt][:, 0:HD], pv[:])
                            for (ps, tab_c, tab_s, QKT, nm) in (
                                    (pq, cqw, sqw, QT, "q"), (pk, ckw, skw, KT, "k")):
                                q32 = s2.tile([128, HD], F32, tag="q32",
                                              name=f"{nm}32_{tt}", bufs=2)
                                nc.vector.tensor_copy(q32[:], ps[:])
                                rinv = rmsnorm_rinv(q32[:], HD, f"{nm}n{tt}")
                                ct = s2.tile([128, HD], BF, tag=f"c{nm}",
                                             name=f"c{nm}{tt}", bufs=2)
                                st_ = s2.tile([128, HD], BF, tag=f"s{nm}",
                                              name=f"s{nm}{tt}", bufs=2)
                                nc.sync.dma_start(
                                    ct[:], tab_c.ap()[tt * 128:(tt + 1) * 128, :])
                                nc.sync.dma_start(
                                    st_[:], tab_s.ap()[tt * 128:(tt + 1) * 128, :])
                                t1 = s2.tile([128, HD], BF, tag="t1",
                                             name=f"t1{nm}{tt}", bufs=2)
                                t2 = s2.tile([128, HD], BF, tag="t2",
                                             name=f"t2{nm}{tt}", bufs=2)
                                nc.vector.scalar_tensor_tensor(
                                    t1[:], q32[:], rinv[:], ct[:],
                                    op0=ALU.mult, op1=ALU.mult)
                                nc.vector.scalar_tensor_tensor(
                                    t2[:].rearrange("p (a b) -> p a b", a=2),
                                    _swap_ap(q32, HALF), rinv[:],
                                    st_[:].rearrange("p (a b) -> p a b", a=2),
                                    op0=ALU.mult, op1=ALU.mult)
                                qr = s2.tile([128, HD], BF, tag="qr",
                                             name=f"qr{nm}{tt}", bufs=2)
                                nc.vector.tensor_add(qr[:], t1[:], t2[:])
                                for h in range(2):
                                    ptr = psP.tile([128, 128], BF, tag="tr", bufs=2,
                                                   name=f"s2t{nm}{tt}_{h}")
                                    nc.tensor.transpose(
                                        ptr[:], qr[:, h * 128:(h + 1) * 128], ident[:])
                                    nc.vector.tensor_copy(
                                        QKT[h][:, tt * 128:(tt + 1) * 128], ptr[:])

                    with tc.tile_pool(name="s3", bufs=2) as s3:
                        for qb in range(4):
                            q0 = 512 * qb
                            probs = {}
                            for i in range(8):
                                kc = q0 - 512 + 128 * i
                                if kc < 0:
                                    continue
                                psc = psP.tile([128, 512], F32, tag="mm", bufs=6,
                                               name=f"psc{qb}_{i}")
                                for h in range(2):
                                    nc.tensor.matmul(psc[:], KT[h][:, kc:kc + 128],
                                                     QT[h][:, q0:q0 + 512],
                                                     start=(h == 0), stop=(h == 1))
                                pr = s3.tile([128, 512], BF, tag="pr",
                                             name=f"pr{qb}_{i}", bufs=10)
                                nc.scalar.activation(pr[:], psc[:], AF.Exp,
                                                     scale=1.0 / 16.0)
                                nc.vector.tensor_mul(pr[:], pr[:], masks[:, i, :])
                                probs[kc] = pr
                            for qs in range(4):
                                qa = q0 + 128 * qs
                                kcs = [kc for kc in range(qa - 512, qa + 128, 128)
                                       if kc >= 0]
                                po = psP.tile([128, HD + 1], F32, tag="mm", bufs=6,
                                              name=f"po{qb}_{qs}")
                                col = qa - q0
                                for j, kc in enumerate(kcs):
                                    nc.tensor.matmul(po[:],
                                                     probs[kc][:, col:col + 128],
                                                     V[kc // 128][:], start=(j == 0),
                                                     stop=(j == len(kcs) - 1))
                                rec = s3.tile([128, 1], F32, tag="rec",
                                              name=f"rec{qb}_{qs}")
                                nc.vector.reciprocal(rec[:], po[:, HD:HD + 1])
                                an = s3.tile([128, HD], BF, tag="an",
                                             name=f"an{qb}_{qs}")
                                nc.vector.tensor_scalar_mul(an[:], po[:, 0:HD], rec[:])
                                for h in range(2):
                                    ptr = psP.tile([128, 128], BF, tag="tr", bufs=2,
                                                   name=f"s3tr{qb}{qs}{h}")
                                    nc.tensor.transpose(
                                        ptr[:], an[:, h * 128:(h + 1) * 128], ident[:])
                                    nc.vector.tensor_copy(aT_sb[h][:, qa:qa + 128],
                                                          ptr[:])
                        # A2A: input shard j = my head's attn^T for token block j;
                        # out rows [256i:256(i+1)) = head i's attn^T for my block.
                        for j in range(NC_):
                            for h in range(2):
                                nc.sync.dma_start(
                                    a2a_in[TS * j + 128 * h:TS * j + 128 * (h + 1), :],
                                    aT_sb[h][:, TS * j:TS * (j + 1)])
                        nc.gpsimd.collective_compute(
                            "AllToAll", ALU.bypass, replica_groups=rg,
                            ins=[a2a_in[:]], outs=[a2a_out[:]])

                # ======== S4: wo + post_attn + residual + pre_ff + AG3 ========
                with tc.tile_pool(name="wpool", bufs=1) as wpool:
                    wo_sb = wpool.tile([128, KA, HID], BF)
                    nc.sync.dma_start(wo_sb[:],
                                      wo_f.ap().rearrange("(k p) n -> p k n", p=128))
                    with tc.tile_pool(name="s4", bufs=2) as s4:
                        w1_pa_b = s4.tile([128, HID], BF, bufs=1)
                        w1_pf_b = s4.tile([128, HID], BF, bufs=1)
                        _bcast_row(nc, w1_pa_b, w1_pa, HID)
                        _bcast_row(nc, w1_pf_b, w1_pf, HID)
                        asl_v = a2a_out[:].rearrange("(k p) t -> p k t", p=128)
                        for t in range(2):
                            asl = s4.tile([128, KA, 128], BF, tag="asl",
                                          name=f"asl{t}", bufs=2)
                            nc.sync.dma_start(asl[:],
                                              asl_v[:, :, t * 128:(t + 1) * 128])
                            ao32 = s4.tile([128, HID], F32, tag="ao32",
                                           name=f"ao32_{t}", bufs=2)
                            for n in range(5):
                                pw = psP.tile([128, 512], F32, tag="mm", bufs=6,
                                              name=f"pw{t}_{n}")
                                for k in range(KA):
                                    nc.tensor.matmul(
                                        pw[:], asl[:, k, :],
                                        wo_sb[:, k, n * 512:(n + 1) * 512],
                                        start=(k == 0), stop=(k == KA - 1))
                                nc.vector.tensor_copy(ao32[:, n * 512:(n + 1) * 512],
                                                      pw[:])
                            rinv_a = rmsnorm_rinv(ao32[:], HID, f"pan{t}")
                            x2 = s4.tile([128, HID], F32, tag="x2", name=f"x2_{t}",
                                         bufs=2)
                            nc.vector.scalar_tensor_tensor(
                                x2[:], ao32[:], rinv_a[:], w1_pa_b[:],
                                op0=ALU.mult, op1=ALU.mult)
                            nc.vector.tensor_add(x2[:], x2[:], x_sb[t][:])
                            nc.sync.dma_start(x2_spill[t * 128:(t + 1) * 128, :], x2[:])
                            rinv_f = rmsnorm_rinv(x2[:], HID, f"pff{t}")
                            h2 = s4.tile([128, HID], BF, tag="h2", name=f"h2_{t}",
                                         bufs=2)
                            nc.vector.scalar_tensor_tensor(
                                h2[:], x2[:], rinv_f[:], w1_pf_b[:],
                                op0=ALU.mult, op1=ALU.mult)
                            h16s[t] = h2
                        for k in range(KH):
                            hTk = s4.tile([128, TS], BF, tag="hTk2",
                                          name=f"hTk2_{k}", bufs=3)
                            for t in range(2):
                                ptr = psP.tile([128, 128], BF, tag="tr", bufs=2,
                                               name=f"s4tr{k}_{t}")
                                nc.tensor.transpose(
                                    ptr[:], h16s[t][:, k * 128:(k + 1) * 128],
                                    ident[:])
                                nc.vector.tensor_copy(hTk[:, t * 128:(t + 1) * 128],
                                                      ptr[:])
                            nc.sync.dma_start(h2T_in[k * 128:(k + 1) * 128, :], hTk[:])
                        nc.gpsimd.collective_compute(
                            "AllGather", ALU.bypass, replica_groups=rg,
                            ins=[h2T_in[:]], outs=[h2T_full[:]])

            # ================= S5: MLP =================
            with tc.tile_pool(name="s5w", bufs=1) as s5w:
                h2T_sb = s5w.tile([128, KH, NC_, TS], BF)
                h2T_fv = h2T_full[:].rearrange("(r k p) t -> r p k t", r=NC_, p=128)
                for r in range(NC_):
                    nc.sync.dma_start(h2T_sb[:, :, r, :], h2T_fv[r])
                actT = [s5w.tile([128, S], BF, name=f"actT{m}") for m in range(MI)]
                with tc.tile_pool(name="s5", bufs=2) as s5:
                    for m in range(MI):
                        wgm = s5.tile([128, KH, 128], BF, tag="wgm",
                                      name=f"wgm{m}", bufs=2)
                        wum = s5.tile([128, KH, 128], BF, tag="wum",
                                      name=f"wum{m}", bufs=2)
                        nc.sync.dma_start(wgm[:], wg_c.ap()[:, m * 128:(m + 1) * 128]
                                          .rearrange("(k p) n -> p k n", p=128))
                        nc.sync.dma_start(wum[:], wu_c.ap()[:, m * 128:(m + 1) * 128]
                                          .rearrange("(k p) n -> p k n", p=128))
                        for r in range(NC_):
                            pg = psP.tile([128, TS], F32, tag="mm", bufs=6,
                                          name=f"pg{m}_{r}")
                            pu = psP.tile([128, TS], F32, tag="mm", bufs=6,
                                          name=f"pu{m}_{r}")
                            for k in range(KH):
                                st, sp = (k == 0), (k == KH - 1)
                                nc.tensor.matmul(pg[:], wgm[:, k, :],
                                                 h2T_sb[:, k, r, :], start=st, stop=sp)
                                nc.tensor.matmul(pu[:], wum[:, k, :],
                                                 h2T_sb[:, k, r, :], start=st, stop=sp)
                            gsc = s5.tile([128, TS], F32, tag="gsc",
                                          name=f"gsc{m}_{r}", bufs=3)
                            nc.scalar.activation(gsc[:], pg[:], AF.Gelu_apprx_tanh)
                            nc.vector.tensor_mul(actT[m][:, r * TS:(r + 1) * TS],
                                                 gsc[:], pu[:])
                    for n in range(5):
                        wdn = s5.tile([128, MI, 512], BF, tag="wdn",
                                      name=f"wdn{n}", bufs=2)
                        nc.sync.dma_start(wdn[:], wd_c.ap()[:, n * 512:(n + 1) * 512]
                                          .rearrange("(i p) n -> p i n", p=128))
                        for tt in range(S // 128):
                            pd = psP.tile([128, 512], F32, tag="mm", bufs=6,
                                          name=f"pd{n}_{tt}")
                            for i in range(MI):
                                nc.tensor.matmul(pd[:],
                                                 actT[i][:, tt * 128:(tt + 1) * 128],
                                                 wdn[:, i, :], start=(i == 0),
                                                 stop=(i == MI - 1))
                            dcp = s5.tile([128, 512], BF, tag="dcp",
                                          name=f"dcp{n}_{tt}", bufs=4)
                            if tt % 2 == 0:
                                nc.vector.tensor_copy(dcp[:], pd[:])
                            else:
                                nc.scalar.activation(dcp[:], pd[:], AF.Copy)
                            nc.sync.dma_start(
                                rs_in[tt * 128:(tt + 1) * 128,
                                      n * 512:(n + 1) * 512], dcp[:])
                    nc.gpsimd.collective_compute(
                        "ReduceScatter", ALU.add, replica_groups=rg,
                        ins=[rs_in[:]], outs=[rs_out[:]])

            # ============ S6: post_ff norm + residual ============
            with tc.tile_pool(name="s6", bufs=2) as s6:
                w1_po_b = s6.tile([128, HID], F32, bufs=1)
                _bcast_row(nc, w1_po_b, w1_po, HID)
                for t in range(2):
                    mlp16 = s6.tile([128, HID], BF, tag="mlp", name=f"mlp{t}", bufs=2)
                    nc.sync.dma_start(mlp16[:], rs_out[:][t * 128:(t + 1) * 128, :])
                    x2l = s6.tile([128, HID], F32, tag="x2l", name=f"x2l{t}", bufs=2)
                    nc.sync.dma_start(x2l[:], x2_spill[t * 128:(t + 1) * 128, :])
                    rinv_o = rmsnorm_rinv(mlp16[:], HID, f"pon{t}")
                    o32 = s6.tile([128, HID], F32, tag="o32", name=f"o32_{t}", bufs=2)
                    nc.vector.scalar_tensor_tensor(o32[:], mlp16[:], rinv_o[:],
                                                   w1_po_b[:], op0=ALU.mult,
                                                   op1=ALU.mult)
                    nc.vector.tensor_add(o32[:], o32[:], x2l[:])
                    nc.sync.dma_start(out_shard.ap()[t * 128:(t + 1) * 128, :], o32[:])

    nc.compile()
    return nc


_NC_CACHE = None


def _get_nc():
    global _NC_CACHE
    if _NC_CACHE is None:
        _NC_CACHE = build_nc()
    return _NC_CACHE


def make_in_maps(hidden_states, position_ids, wq, wk, wv, wo, q_ln_w, k_ln_w,
                 in_ln_w, post_attn_ln_w, pre_ff_ln_w, post_ff_ln_w,
                 w_gate, w_up, w_down):
    bf16 = ml_dtypes.bfloat16
    f32 = np.float32
    x = np.asarray(hidden_states, f32).reshape(S, HID)
    pos = np.asarray(position_ids).reshape(S).astype(np.float64)

    inv_freq = 1.0 / (BASE ** (np.arange(0, HD, 2, dtype=np.float64) / HD))
    freqs = pos[:, None] * inv_freq[None, :]
    emb = np.concatenate([freqs, freqs], axis=1)
    cos = np.cos(emb).astype(f32)
    sin = np.sin(emb).astype(f32)
    w1q = 1.0 + np.asarray(q_ln_w, f32)
    w1k = 1.0 + np.asarray(k_ln_w, f32)

    def rope_tabs(w1):
        w1sw = np.concatenate([w1[HALF:], w1[:HALF]])
        sgn = np.concatenate([-np.ones(HALF, f32), np.ones(HALF, f32)])
        return ((cos * w1[None, :]).astype(bf16),
                (sin * (w1sw * sgn)[None, :]).astype(bf16))

    cqw_np, sqw_np = rope_tabs(w1q)
    ckw_np, skw_np = rope_tabs(w1k)

    wq_r = np.asarray(wq, f32).reshape(HID, NH, HD)
    wk_r = np.asarray(wk, f32).reshape(HID, NKV, HD)
    wv_r = np.asarray(wv, f32).reshape(HID, NKV, HD)
    wg_r = np.asarray(w_gate, f32).reshape(HID, NC_, INTER // NC_)
    wu_r = np.asarray(w_up, f32).reshape(HID, NC_, INTER // NC_)
    wd_r = np.asarray(w_down, f32).reshape(NC_, INTER // NC_, HID)

    common = {
        "wo_f": np.asarray(wo, f32).astype(bf16),
        "w1_in": (1.0 + np.asarray(in_ln_w, f32)).astype(bf16),
        "w1_pa": (1.0 + np.asarray(post_attn_ln_w, f32)).astype(bf16),
        "w1_pf": (1.0 + np.asarray(pre_ff_ln_w, f32)).astype(bf16),
        "w1_po": 1.0 + np.asarray(post_ff_ln_w, f32),
        "cqw": cqw_np, "sqw": sqw_np, "ckw": ckw_np, "skw": skw_np,
    }
    in_maps = []
    for c in range(NC_):
        g = c // (NH // NKV)
        in_maps.append({
            "x_shard": np.ascontiguousarray(x[c * TS:(c + 1) * TS]),
            "wq_c": np.ascontiguousarray(wq_r[:, c, :]).astype(bf16),
            "wk_c": np.ascontiguousarray(wk_r[:, g, :]).astype(bf16),
            "wv_c": np.ascontiguousarray(wv_r[:, g, :]).astype(bf16),
            "wg_c": np.ascontiguousarray(wg_r[:, c, :]).astype(bf16),
            "wu_c": np.ascontiguousarray(wu_r[:, c, :]).astype(bf16),
            "wd_c": np.ascontiguousarray(wd_r[c]).astype(bf16),
            **common,
        })
    return in_maps


def kernel(**inputs):
    in_maps = make_in_maps(**inputs)
    nc = _get_nc()
    res = run_bass_kernel_spmd(nc, in_maps, core_ids=list(range(NC_)))
    out = np.concatenate([res.results[c]["out_shard"] for c in range(NC_)], axis=0)
    return out.reshape(1, S, HID).astype(np.float32)
